# revision 2
# baseline (speedup 1.0000x reference)
"""Binarized 3x3 conv (nn_BiTestConv2d) — TRN2 Bass/Tile kernel.

Reference math:
    bx = sign(x)                                  (forward value of the STE)
    bw = w - mean_o(w);  scale_o = mean|bw|;  binary_w = scale_o * sign(bw)
    y  = conv2d(bx, binary_w, stride 1, pad 1)    (NCHW / OIHW)

Kernel strategy (per core, data-parallel over batch: 32 imgs / 8 cores):
  - weight prep on device: per-out-channel mean & scale (two-stage f32
    reductions), sign -> bf16, PE-transpose to lhsT layout [ci, khw, co]
  - activations: DMA f32, ACT Sign -> bf16 into a zero-padded [128,58,58]
    tile (pad=1 handled by the border zeros)
  - conv as 9 shifted matmuls: K=C_in (2x128), M=C_out tile (128),
    N=448 px (8 output rows); 7 PSUM banks each accumulate 18 matmuls
  - drain: PSUM * scale_o (per-partition scalar) -> SBUF f32 -> DMA out

All matmul operands are exactly +-1 in bf16; PSUM partial sums are exact
integers <= 2304 in f32, so the conv itself is exact; only the final
scale multiply rounds.
"""

import numpy as np

# Problem shapes (hardcoded; the harness calls kernel() with exactly these).
N_FULL, C, H, W = 32, 256, 56, 56
KH, KW = 3, 3
N_CORES = 8
N_IMG = N_FULL // N_CORES  # images per core
P = 128                    # partitions
CI_T = C // P              # 2 input-channel tiles
CO_T = C // P              # 2 output-channel tiles
HP = H + 2                 # padded rows/cols (58)
BLK = 8                    # output rows per PSUM tile
NB = H // BLK              # 7 row-blocks
NPIX_B = BLK * W           # 448 px per block
KFLAT = C * KH * KW        # 2304


def build_module():
    from contextlib import ExitStack

    import concourse.bass as bass  # noqa: F401  (AP helpers)
    import concourse.mybir as mybir
    import concourse.tile as tile
    from concourse import bacc
    from concourse.masks import make_identity

    f32 = mybir.dt.float32
    bf16 = mybir.dt.bfloat16

    nc = bacc.Bacc("TRN2", target_bir_lowering=False, debug=False)

    x_d = nc.dram_tensor("x", [N_IMG, C, H, W], f32, kind="ExternalInput").ap()
    w_d = nc.dram_tensor("weight", [C, C, KH, KW], f32, kind="ExternalInput").ap()
    y_d = nc.dram_tensor("y", [N_IMG, C, H, W], f32, kind="ExternalOutput").ap()

    w_flat = w_d.rearrange("o i kh kw -> o i (kh kw)")  # [256, 256, 9]
    y_flat = y_d.rearrange("n c h w -> n c (h w)")      # [4, 256, 3136]

    with tile.TileContext(nc) as tc, ExitStack() as ctx:
        consts = ctx.enter_context(tc.tile_pool(name="consts", bufs=1))
        wprep = ctx.enter_context(tc.tile_pool(name="wprep", bufs=2))
        xraw_p = ctx.enter_context(tc.tile_pool(name="xraw", bufs=3))
        xpad_p = ctx.enter_context(tc.tile_pool(name="xpad", bufs=4))
        ysb_p = ctx.enter_context(tc.tile_pool(name="ysb", bufs=4))
        psum_p = ctx.enter_context(tc.tile_pool(name="psum", bufs=8, space="PSUM"))

        identity = consts.tile([P, P], bf16)
        make_identity(nc, identity)

        # lhsT weights: [ci(part), ci_tile, khw, co] bf16, +-1
        wT = consts.tile([P, CI_T, KH * KW, C], bf16)
        # per-out-channel scale, column per co_tile: [co(part), co_tile] f32
        scale_sb = consts.tile([P, CO_T], f32)

        # ---------------- weight prep ----------------
        for co_t in range(CO_T):
            o0 = co_t * P
            w_sb = wprep.tile([P, C, KH * KW], f32, tag="w_sb")
            nc.sync.dma_start(out=w_sb, in_=w_flat[o0 : o0 + P])

            # two-stage mean over (i, khw): sums of 9, then sum of 256
            s1 = wprep.tile([P, C], f32, tag="s1")
            nc.vector.reduce_sum(out=s1, in_=w_sb, axis=mybir.AxisListType.X)
            s2 = wprep.tile([P, 1], f32, tag="s2")
            nc.vector.reduce_sum(out=s2, in_=s1, axis=mybir.AxisListType.X)
            mean = wprep.tile([P, 1], f32, tag="mean")
            nc.scalar.mul(out=mean, in_=s2, mul=1.0 / KFLAT)

            bw = wprep.tile([P, C, KH * KW], f32, tag="bw")
            nc.vector.tensor_scalar_sub(out=bw, in0=w_sb, scalar1=mean)

            a1 = wprep.tile([P, C], f32, tag="a1")
            nc.vector.tensor_reduce(
                out=a1,
                in_=bw,
                axis=mybir.AxisListType.X,
                op=mybir.AluOpType.add,
                apply_absolute_value=True,
            )
            a2 = wprep.tile([P, 1], f32, tag="a2")
            nc.vector.reduce_sum(out=a2, in_=a1, axis=mybir.AxisListType.X)
            nc.scalar.mul(out=scale_sb[:, co_t : co_t + 1], in_=a2, mul=1.0 / KFLAT)

            # sign(bw) -> bf16, written permuted to [o(part), khw, i]
            wsign = wprep.tile([P, KH * KW, C], bf16, tag="wsign")
            nc.scalar.sign(out=wsign.rearrange("p khw i -> p i khw"), in_=bw)

            # PE-transpose each [o,128-i] block -> wT[i, khw, o]
            for ci_t in range(CI_T):
                i0 = ci_t * P
                for khw in range(KH * KW):
                    pt = psum_p.tile([P, P], bf16, tag="acc")
                    nc.tensor.transpose(pt, wsign[:, khw, i0 : i0 + P], identity)
                    nc.vector.tensor_copy(
                        out=wT[:, ci_t, khw, o0 : o0 + P], in_=pt
                    )

        # ---------------- conv ----------------
        for img in range(N_IMG):
            xp = []
            for ci_t in range(CI_T):
                i0 = ci_t * P
                xr = xraw_p.tile([P, H, W], f32, tag="xr")
                nc.sync.dma_start(out=xr, in_=x_d[img, i0 : i0 + P])
                t = xpad_p.tile([P, HP, HP], bf16, tag="xp")
                nc.gpsimd.memset(t[:, 0, :], 0.0)
                nc.gpsimd.memset(t[:, HP - 1, :], 0.0)
                nc.gpsimd.memset(t[:, 1 : HP - 1, 0], 0.0)
                nc.gpsimd.memset(t[:, 1 : HP - 1, HP - 1], 0.0)
                nc.scalar.sign(out=t[:, 1 : HP - 1, 1 : HP - 1], in_=xr)
                xp.append(t)

            for co_t in range(CO_T):
                o0 = co_t * P
                psums = [
                    psum_p.tile(
                        [P, NPIX_B], f32, tag="acc", name=f"ps_{img}_{co_t}_{b}"
                    )
                    for b in range(NB)
                ]
                for ci_t in range(CI_T):
                    for khw in range(KH * KW):
                        kh, kw = khw // KW, khw % KW
                        lhsT = wT[:, ci_t, khw, o0 : o0 + P]
                        first = ci_t == 0 and khw == 0
                        last = ci_t == CI_T - 1 and khw == KH * KW - 1
                        for b in range(NB):
                            rhs = xp[ci_t][
                                :, b * BLK + kh : b * BLK + kh + BLK, kw : kw + W
                            ]
                            nc.tensor.matmul(
                                psums[b], lhsT, rhs, start=first, stop=last
                            )
                for b in range(NB):
                    ysb = ysb_p.tile([P, NPIX_B], f32, tag="ysb")
                    nc.vector.tensor_scalar_mul(
                        out=ysb,
                        in0=psums[b],
                        scalar1=scale_sb[:, co_t : co_t + 1],
                    )
                    nc.sync.dma_start(
                        out=y_flat[
                            img, o0 : o0 + P, b * NPIX_B : (b + 1) * NPIX_B
                        ],
                        in_=ysb,
                    )

    nc.compile()
    return nc


_CACHED_NC = None


def kernel(x: np.ndarray, weight: np.ndarray) -> np.ndarray:
    global _CACHED_NC
    from concourse.bass_utils import run_bass_kernel_spmd

    if _CACHED_NC is None:
        _CACHED_NC = build_module()
    nc = _CACHED_NC

    x = np.ascontiguousarray(x, dtype=np.float32)
    weight = np.ascontiguousarray(weight, dtype=np.float32)
    in_maps = [
        {"x": x[c * N_IMG : (c + 1) * N_IMG], "weight": weight}
        for c in range(N_CORES)
    ]
    res = run_bass_kernel_spmd(nc, in_maps, core_ids=list(range(N_CORES)))
    return np.concatenate([r["y"] for r in res.results], axis=0)
